# revision 8
# baseline (speedup 1.0000x reference)
"""GCN aggregation kernel for 8 Trainium2 NeuronCores.

Computes out = leaky_relu(segment_sum(edge_vals * (x @ W^T + b)[edge_cols],
edge_rows)) for a 100K-node, 3.2M-edge graph with D=256.

Strategy (1D destination partitioning, per the vertex-cut hint):
  - Host (untimed): shard destinations across 8 cores (12500 each).  Within
    a core, group edges by (128-dest block, source-slab of 32768 nodes so
    gather indices fit int16), pad each (block, slab) group to 128-edge
    chunks.  x is converted to bf16 and replicated to every core (the
    "all-gather of node features" done host-side).
  - Device, slab-major: for each source slab, dma_gather pulls the bf16
    x rows for all blocks' chunks.  The SWDGE Q7 descriptor-generation rate
    (~8.3 ns/row per queue) is the scarce resource, so gathers round-robin
    over 4 SWDGE queues.  Per chunk, a selector S^T[e,d] = val_e*(dest_e==d)
    feeds one PE matmul psum += S^T.T @ Xg.  S^T comes from two sources to
    balance engines: ~1/4 streamed from host-built bf16 tables over HWDGE,
    ~3/4 built on DVE (iota==dest)*val in one tensor_scalar each.  Each
    (block, slab) PSUM run is folded into a per-block bf16 SBUF accumulator
    (ScalarE copy for the first run, DVE add after).  Per-block epilogue:
    two PE transposes, two matmuls with the replicated W^T, fused
    leaky-relu max(y, 0.01y) on DVE, DMA out.
"""

import os
import sys
from dataclasses import dataclass, field

import numpy as np

for _p in ("/root/.axon_site/_ro/trn_rl_repo", "/opt/trn_rl_repo"):
    if os.path.isdir(_p) and _p not in sys.path:
        sys.path.append(_p)

import ml_dtypes  # noqa: E402  (dependency of jax, present in the image)

import concourse.bacc as bacc  # noqa: E402
import concourse.mybir as mybir  # noqa: E402
import concourse.tile as tile  # noqa: E402
from concourse import bass  # noqa: E402
from concourse.bass_utils import run_bass_kernel_spmd  # noqa: E402

BF16 = ml_dtypes.bfloat16
NEG_SLOPE = 0.01


@dataclass
class Cfg:
    n_nodes: int = 100000
    d: int = 256
    n_cores: int = 8
    slab: int = 32768          # gather-source slab (int16 index limit)
    call_chunks: int = 24      # max 128-edge chunks per dma_gather call
    n_queues: int = 4          # SWDGE queues (ucode max 4)
    stream_every: int = 4      # every Nth call streams S^T from DRAM

    @property
    def dest_per_core(self):
        return self.n_nodes // self.n_cores

    @property
    def nb(self):  # dest blocks per core
        return -(-self.dest_per_core // 128)

    @property
    def ns(self):  # source slabs
        return -(-self.n_nodes // self.slab)

    @property
    def kh(self):  # 128-row halves of the feature dim
        return self.d // 128


@dataclass
class Call:
    t0: int           # first chunk (global index)
    nch: int          # chunks in this call
    stream: bool      # S^T streamed from DRAM (else DVE-built per chunk)


@dataclass
class Chunk:
    t: int            # global chunk index
    b: int            # dest block
    ci: int           # call index within phase
    pos: int          # chunk position within call
    start: bool       # first chunk of the (b, s) PSUM run
    stop: bool        # last chunk of the (b, s) PSUM run
    first_run: bool   # (b, s) is block b's first non-empty run
    last_run: bool    # (b, s) is block b's last non-empty run


@dataclass
class Phase:
    s: int
    slab_lo: int
    slab_rows: int
    calls: list = field(default_factory=list)
    chunks: list = field(default_factory=list)


@dataclass
class Sched:
    T: int
    phases: list
    t_off: np.ndarray        # [nb, ns]
    n_chunks_bs: np.ndarray  # [nb, ns]


def _build_schedule(cfg: Cfg, n_chunks_bs: np.ndarray) -> Sched:
    nb, ns = cfg.nb, cfg.ns
    t_off = np.zeros((nb, ns), dtype=np.int64)
    nonempty = n_chunks_bs > 0
    first_s = np.full(nb, -1)
    last_s = np.full(nb, -1)
    for b in range(nb):
        nz = np.nonzero(nonempty[b])[0]
        if len(nz):
            first_s[b], last_s[b] = nz[0], nz[-1]

    t = 0
    call_counter = 0
    phases = []
    for s in range(ns):
        slab_lo = s * cfg.slab
        slab_rows = min(cfg.slab, cfg.n_nodes - slab_lo)
        ph = Phase(s=s, slab_lo=slab_lo, slab_rows=slab_rows)
        phase_chunks = []  # (b, t, start, stop)
        for b in range(nb):
            ncb = int(n_chunks_bs[b, s])
            if ncb == 0:
                continue
            t_off[b, s] = t
            for j in range(ncb):
                phase_chunks.append((b, t, j == 0, j == ncb - 1))
                t += 1
        for c0 in range(0, len(phase_chunks), cfg.call_chunks):
            seg = phase_chunks[c0 : c0 + cfg.call_chunks]
            ci = len(ph.calls)
            stream = (call_counter % cfg.stream_every) == 0
            call_counter += 1
            ph.calls.append(Call(t0=seg[0][1], nch=len(seg), stream=stream))
            for pos, (b, tg, st_, sp_) in enumerate(seg):
                ph.chunks.append(
                    Chunk(t=tg, b=b, ci=ci, pos=pos, start=st_, stop=sp_,
                          first_run=first_s[b] == s, last_run=last_s[b] == s))
        phases.append(ph)
    return Sched(T=t, phases=phases, t_off=t_off, n_chunks_bs=n_chunks_bs)


def _preprocess(cfg: Cfg, edge_rows, edge_cols, edge_vals):
    """Returns (sched, per_core dicts of descriptor arrays)."""
    nb, ns, dpc = cfg.nb, cfg.ns, cfg.dest_per_core
    rows = np.asarray(edge_rows)
    cols = np.asarray(edge_cols)
    vals = np.asarray(edge_vals, dtype=np.float32)

    core = rows // dpc
    r_loc = rows - core * dpc
    block = r_loc >> 7
    dest_loc = (r_loc & 127).astype(np.int64)
    s_arr = cols // cfg.slab
    c_loc = (cols - s_arr * cfg.slab).astype(np.int16)
    gk = (block * ns + s_arr).astype(np.int64)

    counts = np.bincount(core * (nb * ns) + gk,
                         minlength=cfg.n_cores * nb * ns)
    counts = counts.reshape(cfg.n_cores, nb, ns)
    n_chunks_bs = -(-counts.max(axis=0) // 128)

    sched = _build_schedule(cfg, n_chunks_bs)
    T = sched.T
    slot_base = np.zeros(nb * ns, dtype=np.int64)
    for b in range(nb):
        for s in range(ns):
            slot_base[b * ns + s] = sched.t_off[b, s] * 128

    # columns of streamed calls (for the S^T table); build table only there
    stream_cols = np.zeros(T, dtype=bool)
    for ph in sched.phases:
        for call in ph.calls:
            if call.stream:
                stream_cols[call.t0 : call.t0 + call.nch] = True

    per_core = []
    for k in range(cfg.n_cores):
        sel = core == k
        gk_k = gk[sel]
        order = np.argsort(gk_k, kind="stable")
        gk_s = gk_k[order]
        n_k = gk_s.shape[0]
        uniq, first_idx, cnt = np.unique(gk_s, return_index=True,
                                         return_counts=True)
        pos_in_grp = np.arange(n_k, dtype=np.int64) - np.repeat(first_idx, cnt)
        slots = slot_base[gk_s] + pos_in_grp

        flat_col = np.zeros(T * 128, dtype=np.int16)
        flat_col[slots] = c_loc[sel][order]
        flat_dest = np.zeros(T * 128, dtype=np.float32)
        flat_dest[slots] = dest_loc[sel][order].astype(np.float32)
        flat_val = np.zeros(T * 128, dtype=np.float32)
        flat_val[slots] = vals[sel][order]

        # streamed selector table: st_flat[slot, d] = val where d == dest
        st_flat = np.zeros((T * 128, 128), dtype=BF16)
        st_flat[slots, dest_loc[sel][order]] = vals[sel][order].astype(BF16)
        st_flat = st_flat.reshape(T, 128, 128)
        st_flat[~stream_cols] = 0  # unused columns; keep table exact where read
        st_dram = np.ascontiguousarray(
            st_flat.transpose(1, 0, 2).reshape(128, T * 128))

        idx16 = flat_col.reshape(T * 8, 16).T
        idx_dram = np.tile(idx16, (8, 1))
        per_core.append({
            "idx": np.ascontiguousarray(idx_dram),
            "st": st_dram,
            "dest": np.ascontiguousarray(flat_dest.reshape(T, 128).T),
            "val": np.ascontiguousarray(flat_val.reshape(T, 128).T),
        })
    return sched, per_core


def _build_program(cfg: Cfg, sched: Sched, has_bias: bool):
    dt = mybir.dt
    Alu = mybir.AluOpType
    T = sched.T
    nb = cfg.nb
    nc = bacc.Bacc("TRN2", target_bir_lowering=False, debug=False,
                   num_devices=cfg.n_cores, num_swdge_queues=cfg.n_queues)

    xbf = nc.dram_tensor("xbf", [cfg.n_nodes, cfg.d], dt.bfloat16,
                         kind="ExternalInput")
    idxd = nc.dram_tensor("idx", [128, T * 8], dt.int16, kind="ExternalInput")
    std = nc.dram_tensor("st", [128, T * 128], dt.bfloat16,
                         kind="ExternalInput")
    destd = nc.dram_tensor("dest", [128, T], dt.float32, kind="ExternalInput")
    vald = nc.dram_tensor("val", [128, T], dt.float32, kind="ExternalInput")
    wTd = nc.dram_tensor("wT", [cfg.d, cfg.d], dt.bfloat16,
                         kind="ExternalInput")
    identd = nc.dram_tensor("ident", [128, 128], dt.bfloat16,
                            kind="ExternalInput")
    iotad = nc.dram_tensor("iota", [128, 128], dt.bfloat16,
                           kind="ExternalInput")
    if has_bias:
        bbd = nc.dram_tensor("bb", [128, cfg.d], dt.float32,
                             kind="ExternalInput")
        svd = nc.dram_tensor("sv", [128, nb], dt.float32,
                             kind="ExternalInput")
    outd = nc.dram_tensor("out", [cfg.dest_per_core, cfg.d], dt.float32,
                          kind="ExternalOutput")

    kh = cfg.kh
    qrr = 0
    with tile.TileContext(nc) as tc:
        with (
            tc.tile_pool(name="res", bufs=1) as res,
            tc.tile_pool(name="acc", bufs=nb) as accp,
            tc.tile_pool(name="gath", bufs=4) as gp,
            tc.tile_pool(name="stst", bufs=3) as stsp,
            tc.tile_pool(name="stb", bufs=8) as stbp,
            tc.tile_pool(name="epi", bufs=2 * kh + 2) as ep,
            tc.tile_pool(name="outs", bufs=2) as osp,
            tc.tile_pool(name="psA", bufs=4,
                         space=bass.MemorySpace.PSUM) as psA,
            tc.tile_pool(name="psT", bufs=2,
                         space=bass.MemorySpace.PSUM) as psT,
            tc.tile_pool(name="psO", bufs=2,
                         space=bass.MemorySpace.PSUM) as psO,
        ):
            idx_t = res.tile([128, T * 8], dt.int16)
            nc.sync.dma_start(idx_t[:], idxd[:])
            dest_t = res.tile([128, T], dt.float32)
            nc.sync.dma_start(dest_t[:], destd[:])
            val_t = res.tile([128, T], dt.float32)
            nc.sync.dma_start(val_t[:], vald[:])
            ident_t = res.tile([128, 128], dt.bfloat16)
            nc.sync.dma_start(ident_t[:], identd[:])
            iota_t = res.tile([128, 128], dt.bfloat16)
            nc.sync.dma_start(iota_t[:], iotad[:])
            wt_t = res.tile([128, kh, cfg.d], dt.bfloat16)
            nc.sync.dma_start(
                wt_t[:], wTd.ap().rearrange("(h p) n -> p h n", p=128))
            if has_bias:
                bb_t = res.tile([128, cfg.d], dt.float32)
                nc.sync.dma_start(bb_t[:], bbd[:])
                sv_t = res.tile([128, nb], dt.float32)
                nc.sync.dma_start(sv_t[:], svd[:])

            acc_t = [
                accp.tile([128, cfg.d], dt.bfloat16, tag="acc", name="acc")
                for _ in range(nb)
            ]

            def epilogue(b):
                e_ts = []
                for h in range(kh):
                    tp_ps = psT.tile([128, 128], dt.bfloat16, tag="tp",
                                     name="tp")
                    nc.tensor.transpose(
                        tp_ps[:], acc_t[b][:, h * 128 : h * 128 + 128],
                        ident_t[:])
                    e_t = ep.tile([128, 128], dt.bfloat16, tag="eT",
                                  name="eT", bufs=4)
                    nc.scalar.copy(e_t[:], tp_ps[:])
                    e_ts.append(e_t)
                outp = psO.tile([128, cfg.d], dt.float32, tag="outp",
                                name="outp")
                for h in range(kh):
                    nc.tensor.matmul(
                        outp[:], e_ts[h][:], wt_t[:, h, :],
                        start=(h == 0), stop=(h == kh - 1))
                out_sb = osp.tile([128, cfg.d], dt.float32, tag="outsb",
                                  name="outsb")
                y_sb = ep.tile([128, cfg.d], dt.float32, tag="ysb",
                               name="ysb", bufs=2)
                if has_bias:
                    nc.vector.scalar_tensor_tensor(
                        y_sb[:], bb_t[:], sv_t[:, b : b + 1], outp[:],
                        Alu.mult, Alu.add)
                else:
                    nc.scalar.copy(y_sb[:], outp[:])
                nc.vector.scalar_tensor_tensor(
                    out_sb[:], y_sb[:], NEG_SLOPE, y_sb[:],
                    Alu.mult, Alu.max)
                bs = min(128, cfg.dest_per_core - b * 128)
                nc.sync.dma_start(
                    outd[b * 128 : b * 128 + bs, :], out_sb[:bs, :])

            for ph in sched.phases:
                if not ph.calls:
                    continue
                gtiles, stiles = [], []
                for call in ph.calls:
                    g_t = gp.tile([128, call.nch, cfg.d], dt.bfloat16,
                                  tag="gath", name="gath")
                    nidx = call.nch * 128
                    nc.gpsimd.dma_gather(
                        g_t[:],
                        xbf[ph.slab_lo : ph.slab_lo + ph.slab_rows, :],
                        idx_t[:, call.t0 * 8 : call.t0 * 8 + call.nch * 8],
                        nidx, nidx, cfg.d,
                        single_packet=False,
                        queue_num=qrr % cfg.n_queues)
                    qrr += 1
                    gtiles.append(g_t)
                    if call.stream:
                        s_t = stsp.tile([128, call.nch * 128], dt.bfloat16,
                                        tag="stst", name="stst")
                        nc.sync.dma_start(
                            s_t[:],
                            std[:, call.t0 * 128 : (call.t0 + call.nch) * 128])
                        stiles.append(s_t)
                    else:
                        stiles.append(None)

                run_ps = None
                for ch in ph.chunks:
                    if ch.start:
                        run_ps = psA.tile([128, cfg.d], dt.float32,
                                          tag="runp", name="runp")
                    if stiles[ch.ci] is not None:
                        st_ap = stiles[ch.ci][
                            :, ch.pos * 128 : ch.pos * 128 + 128]
                    else:
                        st_b = stbp.tile([128, 128], dt.bfloat16, tag="stb",
                                         name="stb")
                        nc.vector.tensor_scalar(
                            st_b[:], iota_t[:],
                            dest_t[:, ch.t : ch.t + 1],
                            val_t[:, ch.t : ch.t + 1],
                            Alu.is_equal, Alu.mult)
                        st_ap = st_b[:]
                    nc.tensor.matmul(
                        run_ps[:], st_ap, gtiles[ch.ci][:, ch.pos, :],
                        start=ch.start, stop=ch.stop)
                    if ch.stop:
                        if ch.first_run:
                            nc.scalar.copy(acc_t[ch.b][:], run_ps[:])
                        else:
                            nc.vector.tensor_tensor(
                                acc_t[ch.b][:], acc_t[ch.b][:], run_ps[:],
                                Alu.add)
                        if ch.last_run:
                            epilogue(ch.b)
    nc.compile()
    return nc


def _prepare(cfg: Cfg, x, edge_rows, edge_cols, edge_vals, W_w, W_b):
    """Full host-side prep. Returns (nc, in_maps)."""
    has_bias = bool(np.any(np.asarray(W_b) != 0))
    sched, per_core = _preprocess(cfg, edge_rows, edge_cols, edge_vals)
    nc = _build_program(cfg, sched, has_bias)

    xbf = np.asarray(x, dtype=np.float32).astype(BF16)
    wT = np.ascontiguousarray(np.asarray(W_w, dtype=np.float32).T).astype(BF16)
    ident = np.eye(128, dtype=np.float32).astype(BF16)
    iota = np.tile(np.arange(128, dtype=np.float32).astype(BF16), (128, 1))

    in_maps = []
    for k in range(cfg.n_cores):
        m = {
            "xbf": xbf,
            "idx": per_core[k]["idx"],
            "st": per_core[k]["st"],
            "dest": per_core[k]["dest"],
            "val": per_core[k]["val"],
            "wT": wT,
            "ident": ident,
            "iota": np.ascontiguousarray(iota),
        }
        if has_bias:
            m["bb"] = np.ascontiguousarray(
                np.tile(np.asarray(W_b, dtype=np.float32), (128, 1)))
            sv = np.zeros((128, cfg.nb), dtype=np.float32)
            dpc = cfg.dest_per_core
            sel = (np.asarray(edge_rows) // dpc) == k
            r_loc = np.asarray(edge_rows)[sel] - k * dpc
            np.add.at(
                sv,
                (r_loc & 127, r_loc >> 7),
                np.asarray(edge_vals, dtype=np.float32)[sel],
            )
            m["sv"] = sv
        in_maps.append(m)
    return nc, in_maps


def kernel(x, edge_rows, edge_cols, edge_vals, W_w, W_b, _trace=False):
    cfg = Cfg()
    assert x.shape == (cfg.n_nodes, cfg.d)
    nc, in_maps = _prepare(cfg, x, edge_rows, edge_cols, edge_vals, W_w, W_b)
    res = run_bass_kernel_spmd(nc, in_maps, list(range(cfg.n_cores)),
                               trace=_trace)
    out = np.concatenate(
        [res.results[k]["out"] for k in range(cfg.n_cores)], axis=0)
    if _trace:
        return out.astype(np.float32), res
    return out.astype(np.float32)


# revision 9
# speedup vs baseline: 1.0478x; 1.0478x over previous
"""GCN aggregation kernel for 8 Trainium2 NeuronCores.

Computes out = leaky_relu(segment_sum(edge_vals * (x @ W^T + b)[edge_cols],
edge_rows)) for a 100K-node, 3.2M-edge graph with D=256.

Strategy (1D destination partitioning, per the vertex-cut hint):
  - Host (untimed): shard destinations across 8 cores (12500 each).  Within
    a core, group edges by (128-dest block, source-slab of 32768 nodes so
    gather indices fit int16), pad each (block, slab) group to 128-edge
    chunks.  x is converted to bf16 and replicated to every core (the
    "all-gather of node features" done host-side).
  - Device, slab-major: for each source slab, dma_gather pulls the bf16
    x rows for all blocks' chunks.  The SWDGE Q7 descriptor-generation rate
    (~8.3 ns/row per queue) is the scarce resource, so gathers round-robin
    over 4 SWDGE queues.  Per chunk, a selector S^T[e,d] = val_e*(dest_e==d)
    feeds one PE matmul psum += S^T.T @ Xg.  S^T comes from two sources to
    balance engines: ~1/4 streamed from host-built bf16 tables over HWDGE,
    ~3/4 built on DVE (iota==dest)*val in one tensor_scalar each.  Each
    (block, slab) PSUM run is folded into a per-block bf16 SBUF accumulator
    (ScalarE copy for the first run, DVE add after).  Per-block epilogue:
    two PE transposes, two matmuls with the replicated W^T, fused
    leaky-relu max(y, 0.01y) on DVE, DMA out.
"""

import os
import sys
from dataclasses import dataclass, field

import numpy as np

for _p in ("/root/.axon_site/_ro/trn_rl_repo", "/opt/trn_rl_repo"):
    if os.path.isdir(_p) and _p not in sys.path:
        sys.path.append(_p)

import ml_dtypes  # noqa: E402  (dependency of jax, present in the image)

import concourse.bacc as bacc  # noqa: E402
import concourse.mybir as mybir  # noqa: E402
import concourse.tile as tile  # noqa: E402
from concourse import bass  # noqa: E402
from concourse.bass_utils import run_bass_kernel_spmd  # noqa: E402

BF16 = ml_dtypes.bfloat16
NEG_SLOPE = 0.01


@dataclass
class Cfg:
    n_nodes: int = 100000
    d: int = 256
    n_cores: int = 8
    slab: int = 32768          # gather-source slab (int16 index limit)
    call_chunks: int = 16      # max 128-edge chunks per dma_gather call
    n_queues: int = 4          # SWDGE queues (ucode max 4)
    stream_every: int = 4      # every Nth call streams S^T from DRAM

    @property
    def dest_per_core(self):
        return self.n_nodes // self.n_cores

    @property
    def nb(self):  # dest blocks per core
        return -(-self.dest_per_core // 128)

    @property
    def ns(self):  # source slabs
        return -(-self.n_nodes // self.slab)

    @property
    def kh(self):  # 128-row halves of the feature dim
        return self.d // 128


@dataclass
class Call:
    t0: int           # first chunk (global index)
    nch: int          # chunks in this call
    stream: bool      # S^T streamed from DRAM (else DVE-built per chunk)


@dataclass
class Chunk:
    t: int            # global chunk index
    b: int            # dest block
    ci: int           # call index within phase
    pos: int          # chunk position within call
    start: bool       # first chunk of the (b, s) PSUM run
    stop: bool        # last chunk of the (b, s) PSUM run
    first_run: bool   # (b, s) is block b's first non-empty run
    last_run: bool    # (b, s) is block b's last non-empty run


@dataclass
class Phase:
    s: int
    slab_lo: int
    slab_rows: int
    calls: list = field(default_factory=list)
    chunks: list = field(default_factory=list)


@dataclass
class Sched:
    T: int
    phases: list
    t_off: np.ndarray        # [nb, ns]
    n_chunks_bs: np.ndarray  # [nb, ns]


def _build_schedule(cfg: Cfg, n_chunks_bs: np.ndarray) -> Sched:
    nb, ns = cfg.nb, cfg.ns
    t_off = np.zeros((nb, ns), dtype=np.int64)
    nonempty = n_chunks_bs > 0
    first_s = np.full(nb, -1)
    last_s = np.full(nb, -1)
    for b in range(nb):
        nz = np.nonzero(nonempty[b])[0]
        if len(nz):
            first_s[b], last_s[b] = nz[0], nz[-1]

    t = 0
    call_counter = 0
    phases = []
    for s in range(ns):
        slab_lo = s * cfg.slab
        slab_rows = min(cfg.slab, cfg.n_nodes - slab_lo)
        ph = Phase(s=s, slab_lo=slab_lo, slab_rows=slab_rows)
        phase_chunks = []  # (b, t, start, stop)
        for b in range(nb):
            ncb = int(n_chunks_bs[b, s])
            if ncb == 0:
                continue
            t_off[b, s] = t
            for j in range(ncb):
                phase_chunks.append((b, t, j == 0, j == ncb - 1))
                t += 1
        for c0 in range(0, len(phase_chunks), cfg.call_chunks):
            seg = phase_chunks[c0 : c0 + cfg.call_chunks]
            ci = len(ph.calls)
            stream = (call_counter % cfg.stream_every) == 0
            call_counter += 1
            ph.calls.append(Call(t0=seg[0][1], nch=len(seg), stream=stream))
            for pos, (b, tg, st_, sp_) in enumerate(seg):
                ph.chunks.append(
                    Chunk(t=tg, b=b, ci=ci, pos=pos, start=st_, stop=sp_,
                          first_run=first_s[b] == s, last_run=last_s[b] == s))
        phases.append(ph)
    return Sched(T=t, phases=phases, t_off=t_off, n_chunks_bs=n_chunks_bs)


def _preprocess(cfg: Cfg, edge_rows, edge_cols, edge_vals):
    """Returns (sched, per_core dicts of descriptor arrays)."""
    nb, ns, dpc = cfg.nb, cfg.ns, cfg.dest_per_core
    rows = np.asarray(edge_rows)
    cols = np.asarray(edge_cols)
    vals = np.asarray(edge_vals, dtype=np.float32)

    core = rows // dpc
    r_loc = rows - core * dpc
    block = r_loc >> 7
    dest_loc = (r_loc & 127).astype(np.int64)
    s_arr = cols // cfg.slab
    c_loc = (cols - s_arr * cfg.slab).astype(np.int16)
    gk = (block * ns + s_arr).astype(np.int64)

    counts = np.bincount(core * (nb * ns) + gk,
                         minlength=cfg.n_cores * nb * ns)
    counts = counts.reshape(cfg.n_cores, nb, ns)
    n_chunks_bs = -(-counts.max(axis=0) // 128)

    sched = _build_schedule(cfg, n_chunks_bs)
    T = sched.T
    slot_base = np.zeros(nb * ns, dtype=np.int64)
    for b in range(nb):
        for s in range(ns):
            slot_base[b * ns + s] = sched.t_off[b, s] * 128

    # columns of streamed calls (for the S^T table); build table only there
    stream_cols = np.zeros(T, dtype=bool)
    for ph in sched.phases:
        for call in ph.calls:
            if call.stream:
                stream_cols[call.t0 : call.t0 + call.nch] = True

    per_core = []
    for k in range(cfg.n_cores):
        sel = core == k
        gk_k = gk[sel]
        order = np.argsort(gk_k, kind="stable")
        gk_s = gk_k[order]
        n_k = gk_s.shape[0]
        uniq, first_idx, cnt = np.unique(gk_s, return_index=True,
                                         return_counts=True)
        pos_in_grp = np.arange(n_k, dtype=np.int64) - np.repeat(first_idx, cnt)
        slots = slot_base[gk_s] + pos_in_grp

        flat_col = np.zeros(T * 128, dtype=np.int16)
        flat_col[slots] = c_loc[sel][order]
        flat_dest = np.zeros(T * 128, dtype=np.float32)
        flat_dest[slots] = dest_loc[sel][order].astype(np.float32)
        flat_val = np.zeros(T * 128, dtype=np.float32)
        flat_val[slots] = vals[sel][order]

        # streamed selector table: st_flat[slot, d] = val where d == dest
        st_flat = np.zeros((T * 128, 128), dtype=BF16)
        st_flat[slots, dest_loc[sel][order]] = vals[sel][order].astype(BF16)
        st_flat = st_flat.reshape(T, 128, 128)
        st_flat[~stream_cols] = 0  # unused columns; keep table exact where read
        st_dram = np.ascontiguousarray(
            st_flat.transpose(1, 0, 2).reshape(128, T * 128))

        idx16 = flat_col.reshape(T * 8, 16).T
        idx_dram = np.tile(idx16, (8, 1))
        per_core.append({
            "idx": np.ascontiguousarray(idx_dram),
            "st": st_dram,
            "dest": np.ascontiguousarray(flat_dest.reshape(T, 128).T),
            "val": np.ascontiguousarray(flat_val.reshape(T, 128).T),
        })
    return sched, per_core


def _build_program(cfg: Cfg, sched: Sched, has_bias: bool):
    dt = mybir.dt
    Alu = mybir.AluOpType
    T = sched.T
    nb = cfg.nb
    nc = bacc.Bacc("TRN2", target_bir_lowering=False, debug=False,
                   num_devices=cfg.n_cores, num_swdge_queues=cfg.n_queues)

    xbf = nc.dram_tensor("xbf", [cfg.n_nodes, cfg.d], dt.bfloat16,
                         kind="ExternalInput")
    idxd = nc.dram_tensor("idx", [128, T * 8], dt.int16, kind="ExternalInput")
    std = nc.dram_tensor("st", [128, T * 128], dt.bfloat16,
                         kind="ExternalInput")
    destd = nc.dram_tensor("dest", [128, T], dt.float32, kind="ExternalInput")
    vald = nc.dram_tensor("val", [128, T], dt.float32, kind="ExternalInput")
    wTd = nc.dram_tensor("wT", [cfg.d, cfg.d], dt.bfloat16,
                         kind="ExternalInput")
    identd = nc.dram_tensor("ident", [128, 128], dt.bfloat16,
                            kind="ExternalInput")
    iotad = nc.dram_tensor("iota", [128, 128], dt.bfloat16,
                           kind="ExternalInput")
    if has_bias:
        bbd = nc.dram_tensor("bb", [128, cfg.d], dt.float32,
                             kind="ExternalInput")
        svd = nc.dram_tensor("sv", [128, nb], dt.float32,
                             kind="ExternalInput")
    outd = nc.dram_tensor("out", [cfg.dest_per_core, cfg.d], dt.float32,
                          kind="ExternalOutput")

    kh = cfg.kh
    qrr = 0
    with tile.TileContext(nc) as tc:
        with (
            tc.tile_pool(name="res", bufs=1) as res,
            tc.tile_pool(name="idxp", bufs=2) as idxp,
            tc.tile_pool(name="acc", bufs=nb) as accp,
            tc.tile_pool(name="gath", bufs=8) as gp,
            tc.tile_pool(name="stst", bufs=3) as stsp,
            tc.tile_pool(name="stb", bufs=8) as stbp,
            tc.tile_pool(name="epi", bufs=2 * kh + 2) as ep,
            tc.tile_pool(name="outs", bufs=2) as osp,
            tc.tile_pool(name="psA", bufs=4,
                         space=bass.MemorySpace.PSUM) as psA,
            tc.tile_pool(name="psT", bufs=2,
                         space=bass.MemorySpace.PSUM) as psT,
            tc.tile_pool(name="psO", bufs=2,
                         space=bass.MemorySpace.PSUM) as psO,
        ):
            dest_t = res.tile([128, T], dt.float32)
            nc.sync.dma_start(dest_t[:], destd[:])
            val_t = res.tile([128, T], dt.float32)
            nc.sync.dma_start(val_t[:], vald[:])
            ident_t = res.tile([128, 128], dt.bfloat16)
            nc.sync.dma_start(ident_t[:], identd[:])
            iota_t = res.tile([128, 128], dt.bfloat16)
            nc.sync.dma_start(iota_t[:], iotad[:])
            wt_t = res.tile([128, kh, cfg.d], dt.bfloat16)
            nc.sync.dma_start(
                wt_t[:], wTd.ap().rearrange("(h p) n -> p h n", p=128))
            if has_bias:
                bb_t = res.tile([128, cfg.d], dt.float32)
                nc.sync.dma_start(bb_t[:], bbd[:])
                sv_t = res.tile([128, nb], dt.float32)
                nc.sync.dma_start(sv_t[:], svd[:])

            acc_t = [
                accp.tile([128, cfg.d], dt.bfloat16, tag="acc", name="acc")
                for _ in range(nb)
            ]

            def epilogue(b):
                e_ts = []
                for h in range(kh):
                    tp_ps = psT.tile([128, 128], dt.bfloat16, tag="tp",
                                     name="tp")
                    nc.tensor.transpose(
                        tp_ps[:], acc_t[b][:, h * 128 : h * 128 + 128],
                        ident_t[:])
                    e_t = ep.tile([128, 128], dt.bfloat16, tag="eT",
                                  name="eT", bufs=4)
                    nc.scalar.copy(e_t[:], tp_ps[:])
                    e_ts.append(e_t)
                outp = psO.tile([128, cfg.d], dt.float32, tag="outp",
                                name="outp")
                for h in range(kh):
                    nc.tensor.matmul(
                        outp[:], e_ts[h][:], wt_t[:, h, :],
                        start=(h == 0), stop=(h == kh - 1))
                out_sb = osp.tile([128, cfg.d], dt.float32, tag="outsb",
                                  name="outsb")
                y_sb = ep.tile([128, cfg.d], dt.float32, tag="ysb",
                               name="ysb", bufs=2)
                if has_bias:
                    nc.vector.scalar_tensor_tensor(
                        y_sb[:], bb_t[:], sv_t[:, b : b + 1], outp[:],
                        Alu.mult, Alu.add)
                else:
                    nc.scalar.copy(y_sb[:], outp[:])
                nc.vector.scalar_tensor_tensor(
                    out_sb[:], y_sb[:], NEG_SLOPE, y_sb[:],
                    Alu.mult, Alu.max)
                bs = min(128, cfg.dest_per_core - b * 128)
                nc.sync.dma_start(
                    outd[b * 128 : b * 128 + bs, :], out_sb[:bs, :])

            for ph in sched.phases:
                if not ph.calls:
                    continue
                ph_t0 = ph.calls[0].t0
                ph_nch = sum(c.nch for c in ph.calls)
                idx_t = idxp.tile([128, ph_nch * 8], dt.int16, tag="idx",
                                  name="idx")
                nc.sync.dma_start(
                    idx_t[:], idxd[:, ph_t0 * 8 : (ph_t0 + ph_nch) * 8])
                gtiles, stiles = [], []
                for call in ph.calls:
                    g_t = gp.tile([128, call.nch, cfg.d], dt.bfloat16,
                                  tag="gath", name="gath")
                    nidx = call.nch * 128
                    nc.gpsimd.dma_gather(
                        g_t[:],
                        xbf[ph.slab_lo : ph.slab_lo + ph.slab_rows, :],
                        idx_t[:, (call.t0 - ph_t0) * 8
                              : (call.t0 - ph_t0 + call.nch) * 8],
                        nidx, nidx, cfg.d,
                        single_packet=False,
                        queue_num=qrr % cfg.n_queues)
                    qrr += 1
                    gtiles.append(g_t)
                    if call.stream:
                        s_t = stsp.tile([128, call.nch * 128], dt.bfloat16,
                                        tag="stst", name="stst")
                        nc.sync.dma_start(
                            s_t[:],
                            std[:, call.t0 * 128 : (call.t0 + call.nch) * 128])
                        stiles.append(s_t)
                    else:
                        stiles.append(None)

                run_ps = None
                for ch in ph.chunks:
                    if ch.start:
                        run_ps = psA.tile([128, cfg.d], dt.float32,
                                          tag="runp", name="runp")
                    if stiles[ch.ci] is not None:
                        st_ap = stiles[ch.ci][
                            :, ch.pos * 128 : ch.pos * 128 + 128]
                    else:
                        st_b = stbp.tile([128, 128], dt.bfloat16, tag="stb",
                                         name="stb")
                        nc.vector.tensor_scalar(
                            st_b[:], iota_t[:],
                            dest_t[:, ch.t : ch.t + 1],
                            val_t[:, ch.t : ch.t + 1],
                            Alu.is_equal, Alu.mult)
                        st_ap = st_b[:]
                    nc.tensor.matmul(
                        run_ps[:], st_ap, gtiles[ch.ci][:, ch.pos, :],
                        start=ch.start, stop=ch.stop)
                    if ch.stop:
                        if ch.first_run:
                            nc.scalar.copy(acc_t[ch.b][:], run_ps[:])
                        else:
                            nc.vector.tensor_tensor(
                                acc_t[ch.b][:], acc_t[ch.b][:], run_ps[:],
                                Alu.add)
                        if ch.last_run:
                            epilogue(ch.b)
    nc.compile()
    return nc


def _prepare(cfg: Cfg, x, edge_rows, edge_cols, edge_vals, W_w, W_b):
    """Full host-side prep. Returns (nc, in_maps)."""
    has_bias = bool(np.any(np.asarray(W_b) != 0))
    sched, per_core = _preprocess(cfg, edge_rows, edge_cols, edge_vals)
    nc = _build_program(cfg, sched, has_bias)

    xbf = np.asarray(x, dtype=np.float32).astype(BF16)
    wT = np.ascontiguousarray(np.asarray(W_w, dtype=np.float32).T).astype(BF16)
    ident = np.eye(128, dtype=np.float32).astype(BF16)
    iota = np.tile(np.arange(128, dtype=np.float32).astype(BF16), (128, 1))

    in_maps = []
    for k in range(cfg.n_cores):
        m = {
            "xbf": xbf,
            "idx": per_core[k]["idx"],
            "st": per_core[k]["st"],
            "dest": per_core[k]["dest"],
            "val": per_core[k]["val"],
            "wT": wT,
            "ident": ident,
            "iota": np.ascontiguousarray(iota),
        }
        if has_bias:
            m["bb"] = np.ascontiguousarray(
                np.tile(np.asarray(W_b, dtype=np.float32), (128, 1)))
            sv = np.zeros((128, cfg.nb), dtype=np.float32)
            dpc = cfg.dest_per_core
            sel = (np.asarray(edge_rows) // dpc) == k
            r_loc = np.asarray(edge_rows)[sel] - k * dpc
            np.add.at(
                sv,
                (r_loc & 127, r_loc >> 7),
                np.asarray(edge_vals, dtype=np.float32)[sel],
            )
            m["sv"] = sv
        in_maps.append(m)
    return nc, in_maps


def kernel(x, edge_rows, edge_cols, edge_vals, W_w, W_b, _trace=False):
    cfg = Cfg()
    assert x.shape == (cfg.n_nodes, cfg.d)
    nc, in_maps = _prepare(cfg, x, edge_rows, edge_cols, edge_vals, W_w, W_b)
    res = run_bass_kernel_spmd(nc, in_maps, list(range(cfg.n_cores)),
                               trace=_trace)
    out = np.concatenate(
        [res.results[k]["out"] for k in range(cfg.n_cores)], axis=0)
    if _trace:
        return out.astype(np.float32), res
    return out.astype(np.float32)
